# revision 65
# baseline (speedup 1.0000x reference)
"""Trainium2 Bass kernel for nn_EntropyModel (minGRU LM).

Strategy (8 NeuronCores, data-parallel over batch B=8, one sample per core):

  - Residual stream kept TRANSPOSED on device: hT[d, s] (d on partitions,
    s along free dim), because the minGRU recurrence is computed with the
    DVE `tensor_tensor_scan` instruction (state = a*state + v along the
    free dim, fp32 internal state) which needs lanes on partitions and
    time along free.
  - Layer 1 is token-lookup: rms/hg/nonlinearities of layer 1 depend only
    on the token id (vocab=256), so a1/v1 (scan coefficients/values) and
    h0 (embedding) are precomputed on host as tables and gathered per
    token; the device only runs the scan + out-projection for layer 1.
  - minGRU math:  a = sigmoid(-gate) = 1 - z,  v = z * g(hidden) with
    g(x) = max(x + 0.5, sigmoid(x))  (exact identity for the reference's
    where(x>=0, x+0.5, sigmoid(x))).
  - rmsnorm weight folded into W_hg host-side; layernorm w/b folded into
    the vocab projection: logits = rstd*(W2.T h - mu*colsum(W2)) + b2.
  - Per-token sums over d (rms sumsq, LN mean/meansq) via ones-vector
    matmuls on the PE; per-free broadcast of r[s] via K=1 rank-1 matmuls.
  - Residual add via identity matmul in the PSUM accumulation; the
    per-layer bias is folded into the ACT eviction (bias AP, per
    partition) instead of a rank-1 matmul.
  - Fully per-chunk software pipeline: PRODUCE(li,c) computes hgru(li,c);
    CONSUME(li,c) does the out-projection + eviction + the NEXT layer's
    rms stats/r chain for chunk c (or, for the last layer, the final
    layernorm stats and the vocab projection for chunk c). This removes
    the per-layer-boundary PE stalls and the final-LN tail bubble.
"""

import os
import numpy as np
import ml_dtypes

V, D, L, B, S = 256, 512, 4, 8, 4096
EPS_RMS = 1e-5
EPS_LN = 1e-5
P = 128
KT = D // P            # 4 d-tiles of 128
ET = 2 * D // P        # 8 e-tiles for the hidden/gate projection
SC = 1024              # s-chunk for working tiles
NSC = S // SC
N_CORES = 8

MM_KIND = os.environ.get("EM_MM_KIND", "bf16")
SC_KIND = os.environ.get("EM_SC_KIND", "bf16")

_cache = {}


def _np_dt(kind):
    return np.float32 if kind in ("f32", "f32r") else ml_dtypes.bfloat16


def _build_nc(b2_zero=True):
    import concourse.bass as bass  # noqa: F401
    import concourse.bacc as bacc
    import concourse.mybir as mybir
    import concourse.tile as tile
    from contextlib import ExitStack

    AL = mybir.AluOpType
    AF = mybir.ActivationFunctionType
    F32 = mybir.dt.float32
    MM = {"bf16": mybir.dt.bfloat16, "f32r": mybir.dt.float32r}[MM_KIND]
    SCD = {"bf16": mybir.dt.bfloat16, "f32": mybir.dt.float32}[SC_KIND]

    nc = bacc.Bacc()

    d_h0T = nc.dram_tensor("h0T", [D, S], MM, kind="ExternalInput")
    d_a1T = nc.dram_tensor("a1T", [D, S], SCD, kind="ExternalInput")
    d_v1T = nc.dram_tensor("v1T", [D, S], SCD, kind="ExternalInput")
    d_W1 = nc.dram_tensor("W1", [L - 1, KT, P, 2 * D], MM, kind="ExternalInput")
    d_WoT = nc.dram_tensor("WoT", [L, KT, P, D], MM, kind="ExternalInput")
    d_W2 = nc.dram_tensor("W2", [KT, P, V], MM, kind="ExternalInput")
    d_bout = nc.dram_tensor("boutC", [P, L * KT], F32, kind="ExternalInput")
    d_b2rep = nc.dram_tensor("b2rep", [P, V], F32, kind="ExternalInput")
    d_cw = nc.dram_tensor("cw", [1, V], MM, kind="ExternalInput")
    d_onesk = nc.dram_tensor("onesk", [P, 1], MM, kind="ExternalInput")
    d_onesr = nc.dram_tensor("onesr", [1, P], MM, kind="ExternalInput")
    d_ident = nc.dram_tensor("ident", [P, P], MM, kind="ExternalInput")
    d_out = nc.dram_tensor("out", [S, V], F32, kind="ExternalOutput")

    with ExitStack() as ctx:
        tc = ctx.enter_context(tile.TileContext(nc))
        consts = ctx.enter_context(tc.tile_pool(name="consts", bufs=1))
        hpool = ctx.enter_context(tc.tile_pool(name="hpool", bufs=1))
        work = ctx.enter_context(tc.tile_pool(name="work", bufs=2))
        small = ctx.enter_context(tc.tile_pool(name="small", bufs=1))
        ps_mm = ctx.enter_context(tc.tile_pool(name="ps_mm", bufs=3, space="PSUM"))
        ps_sm = ctx.enter_context(tc.tile_pool(name="ps_sm", bufs=2, space="PSUM"))

        def cdma(name, shape, dt, src, eng=None):
            t = consts.tile(shape, dt, name=name, tag=name)
            (eng or nc.sync).dma_start(out=t, in_=src)
            return t

        # ---------- critical-path-first DMAs: layer-1 chunk streams ----------
        # a/v tiles for the layer-1 scan, chunk 0 first; h0 per chunk.
        h = [hpool.tile([P, S], MM, name=f"h_{k}", tag=f"h_{k}") for k in range(KT)]
        l1_av = {}
        for k in range(KT):
            at = work.tile([P, SC], SCD, name="a_t", tag="a_t", bufs=4)
            nc.sync.dma_start(out=at, in_=d_a1T[k * P:(k + 1) * P, 0:SC])
            vt = work.tile([P, SC], SCD, name="v_t", tag="v_t", bufs=4)
            nc.sync.dma_start(out=vt, in_=d_v1T[k * P:(k + 1) * P, 0:SC])
            l1_av[k, 0] = (at, vt)
        # constants needed by layer-1 consume: issued on the ACT hwdge DMA
        # queue so they land in parallel with the a/v stream on the SP queue
        eps_sb = consts.tile([P, 1], F32, name="eps", tag="eps")
        nc.vector.memset(eps_sb, EPS_RMS)  # EPS_RMS == EPS_LN
        seed_sb = consts.tile([P, 8], mybir.dt.uint32, name="rsqseed",
                              tag="rsqseed")
        nc.vector.memset(seed_sb, 0x5f3759df)
        c15_sb = consts.tile([P, 8], F32, name="c15", tag="c15")
        nc.vector.memset(c15_sb, 1.5)
        half_sb = consts.tile([P, 8], F32, name="halfc", tag="halfc")
        nc.vector.memset(half_sb, 0.5)
        ones8_sb = consts.tile([P, 2, 16], mybir.dt.float8e4, name="ones8",
                               tag="ones8")
        nc.vector.memset(ones8_sb, 1.0)
        # startup consts go on the ACT hwdge queue: it is idle until the
        # first eviction (~16us), so these land in parallel with the a/v
        # stream instead of serializing behind it on the SP queue
        onesk_sb = cdma("onesk", [P, 1], MM, d_onesk[:, :], eng=nc.scalar)
        ident_sb = cdma("ident", [P, P], MM, d_ident[:, :], eng=nc.scalar)
        bout_sb = cdma("boutC", [P, L * KT], F32, d_bout[:, :], eng=nc.scalar)
        onesr_sb = cdma("onesr", [1, P], MM, d_onesr[:, :], eng=nc.scalar)
        wot = {}
        for k in range(KT):
            wot[0, k] = cdma(f"wot_0_{k}", [P, D], MM, d_WoT[0, k],
                             eng=nc.scalar)
        for k in range(KT):
            nc.scalar.dma_start(out=h[k][:, 0:SC],
                                in_=d_h0T[k * P:(k + 1) * P, 0:SC])
        w1 = {}
        for k in range(KT):
            w1[1, k] = cdma(f"w1_1_{k}", [P, 2 * D], MM, d_W1[0, k],
                            eng=nc.scalar)
        ka_q = []

        def keepalive_mm(src_tile):
            """Dummy rank-reduce matmul chained to a freshly-DMA'd tile: keeps
            the PE HAM activity window busy through the DMA-paced layer-0
            stretch so real matmul bursts run at 2.4 GHz, not 1.2."""
            ps = ps_sm.tile([1, 512], F32, name="warm", tag="small")
            nc.tensor.matmul(ps, onesk_sb, src_tile[:, 0:512],
                             start=True, stop=True)

        def load_chunk_inputs(c):
            """DMA a1/v1/h0 for chunk c (c >= 1)."""
            for k in range(KT):
                at = work.tile([P, SC], SCD, name="a_t", tag="a_t", bufs=4)
                nc.sync.dma_start(
                    out=at, in_=d_a1T[k * P:(k + 1) * P, c * SC:(c + 1) * SC])
                vt = work.tile([P, SC], SCD, name="v_t", tag="v_t", bufs=4)
                nc.sync.dma_start(
                    out=vt, in_=d_v1T[k * P:(k + 1) * P, c * SC:(c + 1) * SC])
                l1_av[k, c] = (at, vt)
                ka_q.append(at)
            for k in range(KT):
                nc.sync.dma_start(out=h[k][:, c * SC:(c + 1) * SC],
                                  in_=d_h0T[k * P:(k + 1) * P, c * SC:(c + 1) * SC])

        def load_late_consts_1():
            for k in range(KT):
                wot[1, k] = cdma(f"wot_1_{k}", [P, D], MM, d_WoT[1, k])
                w1[2, k] = cdma(f"w1_2_{k}", [P, 2 * D], MM, d_W1[1, k])

        def load_late_consts_2():
            nonlocal cw_sb, b2_sb, w2sb
            for li in range(2, L):
                for k in range(KT):
                    wot[li, k] = cdma(f"wot_{li}_{k}", [P, D], MM, d_WoT[li, k])
                    if li >= 3:
                        w1[li, k] = cdma(f"w1_{li}_{k}", [P, 2 * D], MM,
                                         d_W1[li - 1, k])
            w2sb = [cdma(f"w2_{k}", [P, V], MM, d_W2[k]) for k in range(KT)]
            cw_sb = cdma("cw", [1, V], MM, d_cw[:, :])
            b2_sb = cdma("b2", [P, V], F32, d_b2rep[:, :])

        cw_sb = b2_sb = w2sb = None

        # per-(layer, chunk) r tiles: separate tiles so a chunk's rb matmul
        # doesn't pick up a whole-tile dependency on later chunks' r writes
        r_row = {(li, c): consts.tile([1, SC], MM, name=f"rrow_{li}_{c}",
                                      tag=f"rrow_{li}_{c}")
                 for li in range(1, L) for c in range(NSC)}
        rt_raw = {li: consts.tile([P, S // P], F32, name=f"rt_{li}",
                                  tag=f"rt_{li}") for li in range(1, L)}
        # final-LN stat tiles (column layout: col = st tile index)
        mt = consts.tile([P, S // P], F32, name="mt", tag="mt")
        qt = consts.tile([P, S // P], F32, name="qt", tag="qt")
        rstd = consts.tile([P, S // P], F32, name="rstd", tag="rstd")
        mmu_row = consts.tile([1, S], MM, name="murow", tag="murow")

        # ---------------- pipeline stages ----------------
        U32 = mybir.dt.uint32

        def pool_rsqrt(out_ap, x_ap, scale, eps):
            """out = 1/sqrt(x*scale + eps), mostly off the DVE.

            Quake-III bit-trick seed + 2 Newton iterations — avoids the ACT
            Sqrt table, whose set excludes Sigmoid and would force a ~1.3us
            table reload twice per chunk. Only the two adjacent seed bit-ops
            run on DVE (Pool's ISA lacks TensorScalar/shift); the rest runs
            on ACT/Pool so no chain op head-of-line blocks the scan-heavy
            DVE FIFO mid-chain.
            """
            ms = small.tile([P, 8], F32, name="ms", tag="ms", bufs=3)
            nc.scalar.activation(ms, x_ap, AF.Identity, bias=eps_sb,
                                 scale=scale)
            y = small.tile([P, 8], F32, name="rsy", tag="rsy", bufs=3)
            # y_bits = 0x5f3759df - (ms_bits >> 1): shift then reversed
            # subtract via a const tile (no u32 wraparound semantics needed)
            nc.vector.tensor_scalar(y.bitcast(U32), ms.bitcast(U32),
                                    1, None, op0=AL.logical_shift_right)
            nc.vector.tensor_sub(y.bitcast(U32), seed_sb, y.bitcast(U32))
            for it in range(2):
                t = small.tile([P, 8], F32, name="rst", tag="rst", bufs=6)
                nc.gpsimd.tensor_mul(t, y, y)
                nc.gpsimd.tensor_mul(t, t, ms)
                nc.gpsimd.tensor_mul(t, t, half_sb)
                nc.gpsimd.tensor_sub(t, c15_sb, t)
                if it == 0:
                    yn = small.tile([P, 8], F32, name="rsy2", tag="rsy", bufs=3)
                    nc.gpsimd.tensor_mul(yn, y, t)
                    y = yn
                else:
                    nc.gpsimd.tensor_mul(out_ap, y, t)

        prev_scan = [None] * KT
        rm_pipe = {}

        def emit_rb_rm(li, c):
            """rb = broadcast r (rank-1 matmul); rm = h * rb."""
            rb = work.tile([P, SC], MM, name="rbc", tag="rbc", bufs=3)
            for sub in range(SC // 512):
                psb = ps_sm.tile([P, 512], F32, name="psb", tag="small")
                s0 = sub * 512
                nc.tensor.matmul(psb, onesr_sb, r_row[li, c][:, s0:s0 + 512],
                                 start=True, stop=True)
                nc.scalar.copy(out=rb[:, sub * 512:(sub + 1) * 512], in_=psb)
            rmst = {}
            for k in range(KT):
                rm = work.tile([P, SC], MM, name="rms", tag="rms", bufs=8)
                nc.vector.tensor_mul(rm, h[k][:, c * SC:(c + 1) * SC], rb)
                rmst[k] = rm
            rm_pipe[li, c] = rmst

        def scan_chunk(k, c, at, vt, split=False):
            init = 0.0 if prev_scan[k] is None else prev_scan[k][:, SC - 1:SC]
            if not split:
                hg_t = work.tile([P, SC], MM, name="hgru", tag="hgru", bufs=9)
                nc.vector.tensor_tensor_scan(
                    hg_t, at, vt, init, op0=AL.mult, op1=AL.add)
                prev_scan[k] = hg_t
                return hg_t
            # final chunk: four 256-wide sub-scans in separate tiles so the
            # last out-projection can consume each quarter as it lands
            parts = []
            for s in range(4):
                pt = work.tile([P, 256], MM, name="hgs", tag="hgs", bufs=16)
                ini = init if s == 0 else parts[-1][:, 255:256]
                nc.vector.tensor_tensor_scan(
                    pt, at[:, s * 256:(s + 1) * 256],
                    vt[:, s * 256:(s + 1) * 256], ini,
                    op0=AL.mult, op1=AL.add)
                parts.append(pt)
            prev_scan[k] = None
            return parts

        def produce(li, c):
            """Compute hgru(li, c): returns {k: hgru_tile}."""
            hgru_c = {}
            if li == 0:
                for k in range(KT):
                    at, vt = l1_av.pop((k, c))
                    hgru_c[k] = scan_chunk(k, c, at, vt)
                return hgru_c
            # rb/rm are pre-emitted one chunk ahead (emit_rb_rm) so the DVE
            # multiplies run before the previous chunk's scans in the FIFO
            if (li, c) not in rm_pipe:
                emit_rb_rm(li, c)
            rmst = rm_pipe.pop((li, c))
            if c + 1 < NSC:
                emit_rb_rm(li, c + 1)
            ght = {}
            for m in [x for k in range(KT) for x in (k, k + KT)]:
                ps = ps_mm.tile([P, SC], F32, name="psh", tag="psmm")
                for sub in range(SC // 512):
                    psl = ps[:, sub * 512:(sub + 1) * 512]
                    for k in range(KT):
                        nc.tensor.matmul(
                            psl, w1[li, k][:, m * P:(m + 1) * P],
                            rmst[k][:, sub * 512:(sub + 1) * 512],
                            start=(k == 0), stop=(k == KT - 1))
                if m < KT:   # hidden half: g = max(hid + 0.5, sigmoid(hid))
                    sg = work.tile([P, SC], SCD, name="sg", tag="sg", bufs=2)
                    nc.scalar.activation(sg, ps, AF.Sigmoid)
                    gh = work.tile([P, SC], SCD, name="gh", tag="gh", bufs=2)
                    nc.vector.scalar_tensor_tensor(
                        gh, ps, 0.5, sg, op0=AL.add, op1=AL.max)
                    ght[m] = gh
                else:        # gate half: z = sigmoid(gate); a = 1-z; v = z*g
                    k = m - KT
                    z = work.tile([P, SC], SCD, name="zt", tag="zt", bufs=2)
                    nc.scalar.activation(z, ps, AF.Sigmoid)
                    at = work.tile([P, SC], SCD, name="a_t", tag="a_t", bufs=4)
                    nc.vector.tensor_scalar(at, z, -1.0, 1.0,
                                            op0=AL.mult, op1=AL.add)
                    vt = work.tile([P, SC], SCD, name="v_t", tag="v_t", bufs=4)
                    nc.vector.tensor_mul(vt, z, ght[k])
                    hgru_c[k] = scan_chunk(
                        k, c, at, vt,
                        split=(li == L - 1 and c == NSC - 1))
            return hgru_c

        def stats_chain(li, c):
            """rms sumsq stats + r for (layer li, chunk c); li in 1..L-1.

            Reads h (post layer li-1), writes r_row[li][0, c*SC:(c+1)*SC].
            """
            # squares in fp8e4, two k-tiles packed per [P, 2, SC] tile:
            # the sumsq matmul then runs as one DoubleRow MM per pair
            # (positive-sum fp8 errors cancel ~1/sqrt(K); ~0.3% on r)
            sqp = []
            for pair in range(2):
                spt = work.tile([P, 2, SC], mybir.dt.float8e4, name="sqp",
                                tag="sqp", bufs=2)
                for j in range(2):
                    k = 2 * pair + j
                    dst = spt[:, j, :]
                    if k % 2 == 0:
                        nc.scalar.activation(
                            dst, h[k][:, c * SC:(c + 1) * SC], AF.Square)
                    else:
                        nc.vector.tensor_mul(dst,
                                             h[k][:, c * SC:(c + 1) * SC],
                                             h[k][:, c * SC:(c + 1) * SC])
                sqp.append(spt)
            for sub in range(SC // 512):
                ps = ps_sm.tile([1, 512], F32, name="pstat", tag="small")
                for pair in range(2):
                    nc.tensor.matmul(
                        ps, ones8_sb[:, :, 0:1],
                        sqp[pair][:, :, sub * 512:(sub + 1) * 512],
                        start=(pair == 0), stop=(pair == 1),
                        perf_mode=mybir.MatmulPerfMode.DoubleRow)
                stg = small.tile([1, 512], F32, name="stg", tag="stage", bufs=2)
                nc.scalar.copy(out=stg, in_=ps)
                # token (1024c + 512 sub + i) lands at [64 sub + i//8, 8c + i%8]
                nc.sync.dma_start(
                    out=rt_raw[li][64 * sub:64 * (sub + 1), 8 * c:8 * (c + 1)],
                    in_=stg)
            # r = 1/sqrt(sumsq/D + eps) on this chunk's 8-column block
            blk = rt_raw[li][:, 8 * c:8 * (c + 1)]
            rrmm = small.tile([P, 8], MM, name="rrmm", tag="rrmm", bufs=3)
            pool_rsqrt(rrmm, blk, 1.0 / D, EPS_RMS)
            # partition-major linearize: token 1024c + 8p + f — identity map
            nc.sync.dma_start(out=r_row[li, c][:, :], in_=rrmm)

        def ln_stats_chain(c):
            """Final layernorm stats (mu/rstd) for chunk c."""
            sqt = {}
            for k in range(KT):
                sq = work.tile([P, SC], MM, name="sqf", tag="sq", bufs=4)
                if k % 2 == 0:
                    nc.scalar.activation(
                        sq, h[k][:, c * SC:(c + 1) * SC], AF.Square)
                else:
                    nc.vector.tensor_mul(sq, h[k][:, c * SC:(c + 1) * SC],
                                         h[k][:, c * SC:(c + 1) * SC])
                sqt[k] = sq
            # the last chunk's chain is latency-exposed: route the psum
            # copies and mu math to DVE so the busy ACT FIFO isn't the tail
            last = c == NSC - 1
            dmae = nc.sync
            for sub in range(SC // 512):
                n = c * (SC // 512) + sub
                psm = ps_sm.tile([1, 512], F32, name="psmu", tag="small")
                for k in range(KT):
                    nc.tensor.matmul(psm, onesk_sb,
                                     h[k][:, n * 512:(n + 1) * 512],
                                     start=(k == 0), stop=(k == KT - 1))
                mm_sl = mmu_row[:, n * 512:(n + 1) * 512]
                stg = small.tile([1, 512], F32, name="stgm", tag="stage", bufs=2)
                nc.scalar.mul(out=mm_sl, in_=psm, mul=-1.0 / D)
                nc.scalar.copy(out=stg, in_=psm)
                for j in range(4):
                    dmae.dma_start(out=mt[:, 4 * n + j:4 * n + j + 1],
                                   in_=stg[:, j * P:(j + 1) * P])
                psq = ps_sm.tile([1, 512], F32, name="psq", tag="small")
                for k in range(KT):
                    nc.tensor.matmul(psq, onesk_sb,
                                     sqt[k][:, sub * 512:(sub + 1) * 512],
                                     start=(k == 0), stop=(k == KT - 1))
                stq = small.tile([1, 512], F32, name="stgq", tag="stage", bufs=2)
                nc.scalar.copy(out=stq, in_=psq)
                for j in range(4):
                    dmae.dma_start(out=qt[:, 4 * n + j:4 * n + j + 1],
                                   in_=stq[:, j * P:(j + 1) * P])
            # rstd = 1/sqrt(qt/D - (mt/D)^2 + eps) for this chunk's 8 st cols
            cs = slice(8 * c, 8 * (c + 1))
            msq = small.tile([P, 8], F32, name="msq", tag="msq", bufs=2)
            nc.scalar.activation(msq, mt[:, cs], AF.Square, scale=1.0 / D)
            var = small.tile([P, 8], F32, name="var", tag="var", bufs=2)
            nc.vector.scalar_tensor_tensor(var, qt[:, cs], 1.0 / D, msq,
                                           op0=AL.mult, op1=AL.subtract)
            pool_rsqrt(rstd[:, cs], var, 1.0, EPS_LN)

        def logits_st(c, sti):
            """One 128-token tile of
            logits[s, v] = rstd[s] * (W2.T h - mu*cw)[s, v] + b2[v]."""
            if True:
                st = c * (SC // P) + sti
                psl = ps_sm.tile([P, V], F32, name="psl", tag="small")
                for k in range(KT):
                    nc.tensor.matmul(psl, h[k][:, st * P:(st + 1) * P],
                                     w2sb[k], start=(k == 0), stop=False)
                nc.tensor.matmul(psl, mmu_row[:, st * P:(st + 1) * P], cw_sb,
                                 start=False, stop=True)
                ot = work.tile([P, V], F32, name="outt", tag="outt", bufs=3)
                if b2_zero and sti % 2 == 0:
                    # alternate evictions ACT/DVE: halves each engine's
                    # serial chain so psl slot reuse doesn't stall the PE
                    # FIFO behind a single saturated queue
                    nc.scalar.activation(ot, psl, AF.Identity,
                                         scale=rstd[:, st:st + 1])
                elif b2_zero:
                    nc.vector.tensor_scalar(ot, psl, rstd[:, st:st + 1],
                                            None, op0=AL.mult)
                else:
                    nc.vector.scalar_tensor_tensor(ot, psl, rstd[:, st:st + 1],
                                                   b2_sb, op0=AL.mult,
                                                   op1=AL.add)
                # gpsimd SWDGE queue: keeps the SP DMA queue free for the
                # latency-critical mt/qt stat reshape DMAs; on the final
                # chunk both queues are idle, so split to halve issue time
                oeng = nc.sync if (c == NSC - 1 and sti % 2 == 1) else nc.gpsimd
                oeng.dma_start(out=d_out[st * P:(st + 1) * P, :], in_=ot)

        def consume(li, c, hgru_c):
            """out-projection for (li, c), eviction with bias, then the next
            layer's stats chain (or the final LN + logits for the last layer).
            """
            split = isinstance(hgru_c[0], list)  # final chunk: 256-wide scans
            sw = 256 if split else 512
            for m in range(KT):
                if li == L - 1 and c >= 1:
                    # vocab projection lags one chunk, interleaved two
                    # st-tiles per out-proj m-tile: the pso matmuls hide the
                    # psl slot-release latency (evictions lag on busy ACT/DVE)
                    logits_st(c - 1, 2 * m)
                    logits_st(c - 1, 2 * m + 1)
                ps = ps_mm.tile([P, SC], F32, name="pso", tag="psmm")
                for sub in range(SC // sw):
                    psl = ps[:, sub * sw:(sub + 1) * sw]
                    base = c * SC + sub * sw
                    for k in range(KT):
                        rhs = (hgru_c[k][sub] if split
                               else hgru_c[k][:, sub * sw:(sub + 1) * sw])
                        nc.tensor.matmul(
                            psl, wot[li, k][:, m * P:(m + 1) * P], rhs,
                            start=(k == 0), stop=False)
                    nc.tensor.matmul(psl, ident_sb,
                                     h[m][:, base:base + sw],
                                     start=False, stop=True)
                bcol = bout_sb[:, li * KT + m:li * KT + m + 1]
                nc.scalar.activation(h[m][:, c * SC:(c + 1) * SC], ps,
                                     AF.Identity, bias=bcol)
            if li < L - 1:
                stats_chain(li + 1, c)
            else:
                ln_stats_chain(c)

        # ---------------- flat software pipeline ----------------
        # warm the PE HAM window through the DMA-bound startup: dummy matmuls
        # chained to early DMA arrivals (one per arrival epoch, so ready real
        # matmuls aren't head-of-line blocked behind a waiting keep-alive)
        for k in range(KT):
            keepalive_mm(l1_av[k, 0][0])
        keepalive_mm(wot[0, 0])
        keepalive_mm(h[0])
        keepalive_mm(w1[1, 0])

        pend = None  # (li, c, hgru_c) awaiting consume
        for li in range(L):
            prev_scan = [None] * KT
            for c in range(NSC):
                if li == 0 and c >= 1:
                    load_chunk_inputs(c)
                hgru_c = produce(li, c)
                if pend is not None:
                    consume(*pend)
                if li == 0 and ka_q:
                    # filler matmuls between out-projection bursts keep HAM hot
                    for t in ka_q:
                        keepalive_mm(t)
                    ka_q.clear()
                pend = (li, c, hgru_c)
                if li == 0 and c == 1:
                    load_late_consts_1()
                elif li == 0 and c == 2:
                    load_late_consts_2()
        consume(*pend)
        for sti in range(SC // P):
            logits_st(NSC - 1, sti)

    nc.finalize()
    return nc


def _sigmoid(x):
    return 1.0 / (1.0 + np.exp(-x))


def _host_prep(inputs):
    """Precompute per-core device inputs (numpy, float64 internal)."""
    mm_np = _np_dt(MM_KIND)
    sc_np = _np_dt(SC_KIND)

    x = np.asarray(inputs["x"]).astype(np.int64)        # [B, S]
    emb = np.asarray(inputs["emb"]).astype(np.float64)  # [V, D]
    rms_w = np.asarray(inputs["rms_w"]).astype(np.float64)
    W_hg = np.asarray(inputs["W_hg"]).astype(np.float64)
    W_out = np.asarray(inputs["W_out"]).astype(np.float64)
    b_out = np.asarray(inputs["b_out"]).astype(np.float64)
    ln_w = np.asarray(inputs["ln_w"]).astype(np.float64)
    ln_b = np.asarray(inputs["ln_b"]).astype(np.float64)
    out_W = np.asarray(inputs["out_W"]).astype(np.float64)
    out_b = np.asarray(inputs["out_b"]).astype(np.float64)

    # layer-1 token tables
    r0 = 1.0 / np.sqrt((emb * emb).mean(-1, keepdims=True) + EPS_RMS)
    rms1 = emb * r0 * rms_w[0][None, :]
    hg1 = rms1 @ W_hg[0].T
    hid1, gate1 = hg1[:, :D], hg1[:, D:]
    z1 = _sigmoid(gate1)
    a_tab = _sigmoid(-gate1)                               # [V, D]
    v_tab = z1 * np.maximum(hid1 + 0.5, _sigmoid(hid1))    # [V, D]

    # folded weights
    W1 = np.stack([(W_hg[i] * rms_w[i][None, :]).T for i in range(1, L)])
    W1 = W1.reshape(L - 1, KT, P, 2 * D)
    WoT = np.stack([W_out[i].T for i in range(L)]).reshape(L, KT, P, D)
    W2 = (out_W * ln_w[None, :]).T                         # [D, V]
    cw = W2.sum(0)[None, :]                                # [1, V]
    b2 = out_b + out_W @ ln_b                              # [V]
    # b_out as per-partition columns: col (li*KT + m) = b_out[li, m*P:(m+1)*P]
    bout_cols = np.ascontiguousarray(
        b_out.reshape(L, KT, P).transpose(2, 0, 1).reshape(P, L * KT))

    common = {
        "W1": W1.astype(mm_np),
        "WoT": WoT.astype(mm_np),
        "W2": W2.reshape(KT, P, V).astype(mm_np),
        "boutC": bout_cols.astype(np.float32),
        "b2rep": np.tile(b2[None, :].astype(np.float32), (P, 1)),
        "cw": cw.astype(mm_np),
        "onesk": np.ones((P, 1), mm_np),
        "onesr": np.ones((1, P), mm_np),
        "ident": np.eye(P, dtype=np.float32).astype(mm_np),
    }
    in_maps = []
    for b in range(B):
        tok = x[b]
        m = dict(common)
        m["h0T"] = np.ascontiguousarray(emb[tok].T).astype(mm_np)
        m["a1T"] = np.ascontiguousarray(a_tab[tok].T).astype(sc_np)
        m["v1T"] = np.ascontiguousarray(v_tab[tok].T).astype(sc_np)
        in_maps.append(m)
    return in_maps


def _get_nc(b2_zero):
    key = ("nc", b2_zero)
    if key not in _cache:
        _cache[key] = _build_nc(b2_zero)
    return _cache[key]


def kernel(**inputs):
    from concourse.bass_utils import run_bass_kernel_spmd

    b2 = (np.asarray(inputs["out_b"], np.float64)
          + np.asarray(inputs["out_W"], np.float64)
          @ np.asarray(inputs["ln_b"], np.float64))
    nc = _get_nc(b2_zero=bool(np.all(b2 == 0.0)))
    in_maps = _host_prep(inputs)
    res = run_bass_kernel_spmd(nc, in_maps, core_ids=list(range(N_CORES)),
                               trace=bool(int(os.environ.get("EM_TRACE", "0"))))
    _cache["last_result"] = res
    out = np.stack([r["out"] for r in res.results], axis=0)
    return out.astype(np.float32)


# revision 66
# speedup vs baseline: 1.0160x; 1.0160x over previous
"""Trainium2 Bass kernel for nn_EntropyModel (minGRU LM).

Strategy (8 NeuronCores, data-parallel over batch B=8, one sample per core):

  - Residual stream kept TRANSPOSED on device: hT[d, s] (d on partitions,
    s along free dim), because the minGRU recurrence is computed with the
    DVE `tensor_tensor_scan` instruction (state = a*state + v along the
    free dim, fp32 internal state) which needs lanes on partitions and
    time along free.
  - Layer 1 is token-lookup: rms/hg/nonlinearities of layer 1 depend only
    on the token id (vocab=256), so a1/v1 (scan coefficients/values) and
    h0 (embedding) are precomputed on host as tables and gathered per
    token; the device only runs the scan + out-projection for layer 1.
  - minGRU math:  a = sigmoid(-gate) = 1 - z,  v = z * g(hidden) with
    g(x) = max(x + 0.5, sigmoid(x))  (exact identity for the reference's
    where(x>=0, x+0.5, sigmoid(x))).
  - rmsnorm weight folded into W_hg host-side; layernorm w/b folded into
    the vocab projection: logits = rstd*(W2.T h - mu*colsum(W2)) + b2.
  - Per-token sums over d (rms sumsq, LN mean/meansq) via ones-vector
    matmuls on the PE; per-free broadcast of r[s] via K=1 rank-1 matmuls.
  - Residual add via identity matmul in the PSUM accumulation; the
    per-layer bias is folded into the ACT eviction (bias AP, per
    partition) instead of a rank-1 matmul.
  - Fully per-chunk software pipeline: PRODUCE(li,c) computes hgru(li,c);
    CONSUME(li,c) does the out-projection + eviction + the NEXT layer's
    rms stats/r chain for chunk c (or, for the last layer, the final
    layernorm stats and the vocab projection for chunk c). This removes
    the per-layer-boundary PE stalls and the final-LN tail bubble.
"""

import os
import numpy as np
import ml_dtypes

V, D, L, B, S = 256, 512, 4, 8, 4096
EPS_RMS = 1e-5
EPS_LN = 1e-5
P = 128
KT = D // P            # 4 d-tiles of 128
ET = 2 * D // P        # 8 e-tiles for the hidden/gate projection
SC = 1024              # s-chunk for working tiles
NSC = S // SC
N_CORES = 8

MM_KIND = os.environ.get("EM_MM_KIND", "bf16")
SC_KIND = os.environ.get("EM_SC_KIND", "bf16")

_cache = {}


def _np_dt(kind):
    return np.float32 if kind in ("f32", "f32r") else ml_dtypes.bfloat16


def _build_nc(b2_zero=True):
    import concourse.bass as bass  # noqa: F401
    import concourse.bacc as bacc
    import concourse.mybir as mybir
    import concourse.tile as tile
    from contextlib import ExitStack

    AL = mybir.AluOpType
    AF = mybir.ActivationFunctionType
    F32 = mybir.dt.float32
    MM = {"bf16": mybir.dt.bfloat16, "f32r": mybir.dt.float32r}[MM_KIND]
    SCD = {"bf16": mybir.dt.bfloat16, "f32": mybir.dt.float32}[SC_KIND]

    nc = bacc.Bacc()

    d_h0T = nc.dram_tensor("h0T", [D, S], MM, kind="ExternalInput")
    d_a1T = nc.dram_tensor("a1T", [D, S], SCD, kind="ExternalInput")
    d_v1T = nc.dram_tensor("v1T", [D, S], SCD, kind="ExternalInput")
    d_W1 = nc.dram_tensor("W1", [L - 1, KT, P, 2 * D], MM, kind="ExternalInput")
    d_WoT = nc.dram_tensor("WoT", [L, KT, P, D], MM, kind="ExternalInput")
    d_W2 = nc.dram_tensor("W2", [KT, P, V], MM, kind="ExternalInput")
    d_bout = nc.dram_tensor("boutC", [P, L * KT], F32, kind="ExternalInput")
    d_b2rep = nc.dram_tensor("b2rep", [P, V], F32, kind="ExternalInput")
    d_cw = nc.dram_tensor("cw", [1, V], MM, kind="ExternalInput")
    d_onesk = nc.dram_tensor("onesk", [P, 1], MM, kind="ExternalInput")
    d_onesr = nc.dram_tensor("onesr", [1, P], MM, kind="ExternalInput")
    d_ident = nc.dram_tensor("ident", [P, P], MM, kind="ExternalInput")
    d_out = nc.dram_tensor("out", [S, V], F32, kind="ExternalOutput")

    with ExitStack() as ctx:
        tc = ctx.enter_context(tile.TileContext(nc))
        consts = ctx.enter_context(tc.tile_pool(name="consts", bufs=1))
        hpool = ctx.enter_context(tc.tile_pool(name="hpool", bufs=1))
        work = ctx.enter_context(tc.tile_pool(name="work", bufs=2))
        small = ctx.enter_context(tc.tile_pool(name="small", bufs=1))
        ps_mm = ctx.enter_context(tc.tile_pool(name="ps_mm", bufs=3, space="PSUM"))
        ps_sm = ctx.enter_context(tc.tile_pool(name="ps_sm", bufs=2, space="PSUM"))

        def cdma(name, shape, dt, src, eng=None):
            t = consts.tile(shape, dt, name=name, tag=name)
            (eng or nc.sync).dma_start(out=t, in_=src)
            return t

        # ---------- critical-path-first DMAs: layer-1 chunk streams ----------
        # a/v tiles for the layer-1 scan, chunk 0 first; h0 per chunk.
        h = [hpool.tile([P, S], MM, name=f"h_{k}", tag=f"h_{k}") for k in range(KT)]
        l1_av = {}
        for k in range(KT):
            at = work.tile([P, SC], SCD, name="a_t", tag="a_t", bufs=4)
            nc.sync.dma_start(out=at, in_=d_a1T[k * P:(k + 1) * P, 0:SC])
            vt = work.tile([P, SC], SCD, name="v_t", tag="v_t", bufs=4)
            nc.sync.dma_start(out=vt, in_=d_v1T[k * P:(k + 1) * P, 0:SC])
            l1_av[k, 0] = (at, vt)
        # constants needed by layer-1 consume: issued on the ACT hwdge DMA
        # queue so they land in parallel with the a/v stream on the SP queue
        eps_sb = consts.tile([P, 1], F32, name="eps", tag="eps")
        nc.vector.memset(eps_sb, EPS_RMS)  # EPS_RMS == EPS_LN
        seed_sb = consts.tile([P, 8], mybir.dt.uint32, name="rsqseed",
                              tag="rsqseed")
        nc.vector.memset(seed_sb, 0x5f3759df)
        c15_sb = consts.tile([P, 8], F32, name="c15", tag="c15")
        nc.vector.memset(c15_sb, 1.5)
        half_sb = consts.tile([P, 8], F32, name="halfc", tag="halfc")
        nc.vector.memset(half_sb, 0.5)
        # startup consts go on the ACT hwdge queue: it is idle until the
        # first eviction (~16us), so these land in parallel with the a/v
        # stream instead of serializing behind it on the SP queue
        onesk_sb = cdma("onesk", [P, 1], MM, d_onesk[:, :], eng=nc.scalar)
        ident_sb = cdma("ident", [P, P], MM, d_ident[:, :], eng=nc.scalar)
        bout_sb = cdma("boutC", [P, L * KT], F32, d_bout[:, :], eng=nc.scalar)
        onesr_sb = cdma("onesr", [1, P], MM, d_onesr[:, :], eng=nc.scalar)
        wot = {}
        for k in range(KT):
            wot[0, k] = cdma(f"wot_0_{k}", [P, D], MM, d_WoT[0, k],
                             eng=nc.scalar)
        for k in range(KT):
            nc.scalar.dma_start(out=h[k][:, 0:SC],
                                in_=d_h0T[k * P:(k + 1) * P, 0:SC])
        w1 = {}
        for k in range(KT):
            w1[1, k] = cdma(f"w1_1_{k}", [P, 2 * D], MM, d_W1[0, k],
                            eng=nc.scalar)
        ka_q = []

        def keepalive_mm(src_tile):
            """Dummy rank-reduce matmul chained to a freshly-DMA'd tile: keeps
            the PE HAM activity window busy through the DMA-paced layer-0
            stretch so real matmul bursts run at 2.4 GHz, not 1.2."""
            ps = ps_sm.tile([1, 512], F32, name="warm", tag="small")
            nc.tensor.matmul(ps, onesk_sb, src_tile[:, 0:512],
                             start=True, stop=True)

        def load_chunk_inputs(c):
            """DMA a1/v1/h0 for chunk c (c >= 1)."""
            for k in range(KT):
                at = work.tile([P, SC], SCD, name="a_t", tag="a_t", bufs=4)
                nc.sync.dma_start(
                    out=at, in_=d_a1T[k * P:(k + 1) * P, c * SC:(c + 1) * SC])
                vt = work.tile([P, SC], SCD, name="v_t", tag="v_t", bufs=4)
                nc.sync.dma_start(
                    out=vt, in_=d_v1T[k * P:(k + 1) * P, c * SC:(c + 1) * SC])
                l1_av[k, c] = (at, vt)
                ka_q.append(at)
            for k in range(KT):
                nc.sync.dma_start(out=h[k][:, c * SC:(c + 1) * SC],
                                  in_=d_h0T[k * P:(k + 1) * P, c * SC:(c + 1) * SC])

        def load_late_consts_1():
            for k in range(KT):
                wot[1, k] = cdma(f"wot_1_{k}", [P, D], MM, d_WoT[1, k])
                w1[2, k] = cdma(f"w1_2_{k}", [P, 2 * D], MM, d_W1[1, k])

        def load_late_consts_2():
            nonlocal cw_sb, b2_sb, w2sb
            for li in range(2, L):
                for k in range(KT):
                    wot[li, k] = cdma(f"wot_{li}_{k}", [P, D], MM, d_WoT[li, k])
                    if li >= 3:
                        w1[li, k] = cdma(f"w1_{li}_{k}", [P, 2 * D], MM,
                                         d_W1[li - 1, k])
            w2sb = [cdma(f"w2_{k}", [P, V], MM, d_W2[k]) for k in range(KT)]
            cw_sb = cdma("cw", [1, V], MM, d_cw[:, :])
            b2_sb = cdma("b2", [P, V], F32, d_b2rep[:, :])

        cw_sb = b2_sb = w2sb = None

        # per-(layer, chunk) r tiles: separate tiles so a chunk's rb matmul
        # doesn't pick up a whole-tile dependency on later chunks' r writes
        r_row = {(li, c): consts.tile([1, SC], MM, name=f"rrow_{li}_{c}",
                                      tag=f"rrow_{li}_{c}")
                 for li in range(1, L) for c in range(NSC)}
        rt_raw = {li: consts.tile([P, S // P], F32, name=f"rt_{li}",
                                  tag=f"rt_{li}") for li in range(1, L)}
        # final-LN stat tiles (column layout: col = st tile index)
        mt = consts.tile([P, S // P], F32, name="mt", tag="mt")
        qt = consts.tile([P, S // P], F32, name="qt", tag="qt")
        rstd = consts.tile([P, S // P], F32, name="rstd", tag="rstd")
        mmu_row = consts.tile([1, S], MM, name="murow", tag="murow")

        # ---------------- pipeline stages ----------------
        U32 = mybir.dt.uint32

        def pool_rsqrt(out_ap, x_ap, scale, eps):
            """out = 1/sqrt(x*scale + eps), mostly off the DVE.

            Quake-III bit-trick seed + 2 Newton iterations — avoids the ACT
            Sqrt table, whose set excludes Sigmoid and would force a ~1.3us
            table reload twice per chunk. Only the two adjacent seed bit-ops
            run on DVE (Pool's ISA lacks TensorScalar/shift); the rest runs
            on ACT/Pool so no chain op head-of-line blocks the scan-heavy
            DVE FIFO mid-chain.
            """
            ms = small.tile([P, 8], F32, name="ms", tag="ms", bufs=3)
            nc.scalar.activation(ms, x_ap, AF.Identity, bias=eps_sb,
                                 scale=scale)
            y = small.tile([P, 8], F32, name="rsy", tag="rsy", bufs=3)
            # y_bits = 0x5f3759df - (ms_bits >> 1): shift then reversed
            # subtract via a const tile (no u32 wraparound semantics needed)
            nc.vector.tensor_scalar(y.bitcast(U32), ms.bitcast(U32),
                                    1, None, op0=AL.logical_shift_right)
            nc.vector.tensor_sub(y.bitcast(U32), seed_sb, y.bitcast(U32))
            for it in range(2):
                t = small.tile([P, 8], F32, name="rst", tag="rst", bufs=6)
                nc.gpsimd.tensor_mul(t, y, y)
                nc.gpsimd.tensor_mul(t, t, ms)
                nc.gpsimd.tensor_mul(t, t, half_sb)
                nc.gpsimd.tensor_sub(t, c15_sb, t)
                if it == 0:
                    yn = small.tile([P, 8], F32, name="rsy2", tag="rsy", bufs=3)
                    nc.gpsimd.tensor_mul(yn, y, t)
                    y = yn
                else:
                    nc.gpsimd.tensor_mul(out_ap, y, t)

        prev_scan = [None] * KT
        rm_pipe = {}

        def emit_rb_rm(li, c):
            """rb = broadcast r (rank-1 matmul); rm = h * rb."""
            rb = work.tile([P, SC], MM, name="rbc", tag="rbc", bufs=3)
            for sub in range(SC // 512):
                psb = ps_sm.tile([P, 512], F32, name="psb", tag="small")
                s0 = sub * 512
                nc.tensor.matmul(psb, onesr_sb, r_row[li, c][:, s0:s0 + 512],
                                 start=True, stop=True)
                nc.scalar.copy(out=rb[:, sub * 512:(sub + 1) * 512], in_=psb)
            rmst = {}
            for k in range(KT):
                rm = work.tile([P, SC], MM, name="rms", tag="rms", bufs=8)
                nc.vector.tensor_mul(rm, h[k][:, c * SC:(c + 1) * SC], rb)
                rmst[k] = rm
            rm_pipe[li, c] = rmst

        def scan_chunk(k, c, at, vt, split=False):
            init = 0.0 if prev_scan[k] is None else prev_scan[k][:, SC - 1:SC]
            if not split:
                hg_t = work.tile([P, SC], MM, name="hgru", tag="hgru", bufs=9)
                nc.vector.tensor_tensor_scan(
                    hg_t, at, vt, init, op0=AL.mult, op1=AL.add)
                prev_scan[k] = hg_t
                return hg_t
            # final chunk: four 256-wide sub-scans in separate tiles so the
            # last out-projection can consume each quarter as it lands
            parts = []
            for s in range(4):
                pt = work.tile([P, 256], MM, name="hgs", tag="hgs", bufs=16)
                ini = init if s == 0 else parts[-1][:, 255:256]
                nc.vector.tensor_tensor_scan(
                    pt, at[:, s * 256:(s + 1) * 256],
                    vt[:, s * 256:(s + 1) * 256], ini,
                    op0=AL.mult, op1=AL.add)
                parts.append(pt)
            prev_scan[k] = None
            return parts

        def produce(li, c):
            """Compute hgru(li, c): returns {k: hgru_tile}."""
            hgru_c = {}
            if li == 0:
                for k in range(KT):
                    at, vt = l1_av.pop((k, c))
                    hgru_c[k] = scan_chunk(k, c, at, vt)
                return hgru_c
            # rb/rm are pre-emitted one chunk ahead (emit_rb_rm) so the DVE
            # multiplies run before the previous chunk's scans in the FIFO
            if (li, c) not in rm_pipe:
                emit_rb_rm(li, c)
            rmst = rm_pipe.pop((li, c))
            if c + 1 < NSC:
                emit_rb_rm(li, c + 1)
            ght = {}
            for m in [x for k in range(KT) for x in (k, k + KT)]:
                ps = ps_mm.tile([P, SC], F32, name="psh", tag="psmm")
                for sub in range(SC // 512):
                    psl = ps[:, sub * 512:(sub + 1) * 512]
                    for k in range(KT):
                        nc.tensor.matmul(
                            psl, w1[li, k][:, m * P:(m + 1) * P],
                            rmst[k][:, sub * 512:(sub + 1) * 512],
                            start=(k == 0), stop=(k == KT - 1))
                if m < KT:   # hidden half: g = max(hid + 0.5, sigmoid(hid))
                    sg = work.tile([P, SC], SCD, name="sg", tag="sg", bufs=3)
                    nc.scalar.activation(sg, ps, AF.Sigmoid)
                    gh = work.tile([P, SC], SCD, name="gh", tag="gh", bufs=3)
                    nc.vector.scalar_tensor_tensor(
                        gh, ps, 0.5, sg, op0=AL.add, op1=AL.max)
                    ght[m] = gh
                else:        # gate half: z = sigmoid(gate); a = 1-z; v = z*g
                    k = m - KT
                    z = work.tile([P, SC], SCD, name="zt", tag="zt", bufs=2)
                    nc.scalar.activation(z, ps, AF.Sigmoid)
                    at = work.tile([P, SC], SCD, name="a_t", tag="a_t", bufs=4)
                    nc.vector.tensor_scalar(at, z, -1.0, 1.0,
                                            op0=AL.mult, op1=AL.add)
                    vt = work.tile([P, SC], SCD, name="v_t", tag="v_t", bufs=4)
                    nc.vector.tensor_mul(vt, z, ght[k])
                    hgru_c[k] = scan_chunk(
                        k, c, at, vt,
                        split=(li == L - 1 and c == NSC - 1))
            return hgru_c

        def stats_chain(li, c):
            """rms sumsq stats + r for (layer li, chunk c); li in 1..L-1.

            Reads h (post layer li-1), writes r_row[li][0, c*SC:(c+1)*SC].
            """
            sqt = {}
            for k in range(KT):
                sq = work.tile([P, SC], MM, name="sq", tag="sq", bufs=4)
                if k % 2 == 0:
                    nc.scalar.activation(
                        sq, h[k][:, c * SC:(c + 1) * SC], AF.Square)
                else:
                    nc.vector.tensor_mul(sq, h[k][:, c * SC:(c + 1) * SC],
                                         h[k][:, c * SC:(c + 1) * SC])
                sqt[k] = sq
            for sub in range(SC // 512):
                ps = ps_sm.tile([1, 512], F32, name="pstat", tag="small")
                for k in range(KT):
                    nc.tensor.matmul(
                        ps, onesk_sb, sqt[k][:, sub * 512:(sub + 1) * 512],
                        start=(k == 0), stop=(k == KT - 1))
                stg = small.tile([1, 512], F32, name="stg", tag="stage", bufs=2)
                nc.scalar.copy(out=stg, in_=ps)
                # token (1024c + 512 sub + i) lands at [64 sub + i//8, 8c + i%8]
                nc.sync.dma_start(
                    out=rt_raw[li][64 * sub:64 * (sub + 1), 8 * c:8 * (c + 1)],
                    in_=stg)
            # r = 1/sqrt(sumsq/D + eps) on this chunk's 8-column block
            blk = rt_raw[li][:, 8 * c:8 * (c + 1)]
            rrmm = small.tile([P, 8], MM, name="rrmm", tag="rrmm", bufs=3)
            pool_rsqrt(rrmm, blk, 1.0 / D, EPS_RMS)
            # partition-major linearize: token 1024c + 8p + f — identity map
            nc.sync.dma_start(out=r_row[li, c][:, :], in_=rrmm)

        def ln_stats_chain(c):
            """Final layernorm stats (mu/rstd) for chunk c."""
            sqt = {}
            for k in range(KT):
                sq = work.tile([P, SC], MM, name="sqf", tag="sq", bufs=4)
                if k % 2 == 0:
                    nc.scalar.activation(
                        sq, h[k][:, c * SC:(c + 1) * SC], AF.Square)
                else:
                    nc.vector.tensor_mul(sq, h[k][:, c * SC:(c + 1) * SC],
                                         h[k][:, c * SC:(c + 1) * SC])
                sqt[k] = sq
            # the last chunk's chain is latency-exposed: route the psum
            # copies and mu math to DVE so the busy ACT FIFO isn't the tail
            last = c == NSC - 1
            dmae = nc.sync
            for sub in range(SC // 512):
                n = c * (SC // 512) + sub
                psm = ps_sm.tile([1, 512], F32, name="psmu", tag="small")
                for k in range(KT):
                    nc.tensor.matmul(psm, onesk_sb,
                                     h[k][:, n * 512:(n + 1) * 512],
                                     start=(k == 0), stop=(k == KT - 1))
                mm_sl = mmu_row[:, n * 512:(n + 1) * 512]
                stg = small.tile([1, 512], F32, name="stgm", tag="stage", bufs=2)
                nc.scalar.mul(out=mm_sl, in_=psm, mul=-1.0 / D)
                nc.scalar.copy(out=stg, in_=psm)
                for j in range(4):
                    dmae.dma_start(out=mt[:, 4 * n + j:4 * n + j + 1],
                                   in_=stg[:, j * P:(j + 1) * P])
                psq = ps_sm.tile([1, 512], F32, name="psq", tag="small")
                for k in range(KT):
                    nc.tensor.matmul(psq, onesk_sb,
                                     sqt[k][:, sub * 512:(sub + 1) * 512],
                                     start=(k == 0), stop=(k == KT - 1))
                stq = small.tile([1, 512], F32, name="stgq", tag="stage", bufs=2)
                nc.scalar.copy(out=stq, in_=psq)
                for j in range(4):
                    dmae.dma_start(out=qt[:, 4 * n + j:4 * n + j + 1],
                                   in_=stq[:, j * P:(j + 1) * P])
            # rstd = 1/sqrt(qt/D - (mt/D)^2 + eps) for this chunk's 8 st cols
            cs = slice(8 * c, 8 * (c + 1))
            msq = small.tile([P, 8], F32, name="msq", tag="msq", bufs=2)
            nc.scalar.activation(msq, mt[:, cs], AF.Square, scale=1.0 / D)
            var = small.tile([P, 8], F32, name="var", tag="var", bufs=2)
            nc.vector.scalar_tensor_tensor(var, qt[:, cs], 1.0 / D, msq,
                                           op0=AL.mult, op1=AL.subtract)
            pool_rsqrt(rstd[:, cs], var, 1.0, EPS_LN)

        def logits_st(c, sti):
            """One 128-token tile of
            logits[s, v] = rstd[s] * (W2.T h - mu*cw)[s, v] + b2[v]."""
            if True:
                st = c * (SC // P) + sti
                psl = ps_sm.tile([P, V], F32, name="psl", tag="small")
                for k in range(KT):
                    nc.tensor.matmul(psl, h[k][:, st * P:(st + 1) * P],
                                     w2sb[k], start=(k == 0), stop=False)
                nc.tensor.matmul(psl, mmu_row[:, st * P:(st + 1) * P], cw_sb,
                                 start=False, stop=True)
                ot = work.tile([P, V], F32, name="outt", tag="outt", bufs=3)
                if b2_zero and sti % 2 == 0:
                    # alternate evictions ACT/DVE: halves each engine's
                    # serial chain so psl slot reuse doesn't stall the PE
                    # FIFO behind a single saturated queue
                    nc.scalar.activation(ot, psl, AF.Identity,
                                         scale=rstd[:, st:st + 1])
                elif b2_zero:
                    nc.vector.tensor_scalar(ot, psl, rstd[:, st:st + 1],
                                            None, op0=AL.mult)
                else:
                    nc.vector.scalar_tensor_tensor(ot, psl, rstd[:, st:st + 1],
                                                   b2_sb, op0=AL.mult,
                                                   op1=AL.add)
                # gpsimd SWDGE queue: keeps the SP DMA queue free for the
                # latency-critical mt/qt stat reshape DMAs; on the final
                # chunk both queues are idle, so split to halve issue time
                oeng = nc.sync if (c == NSC - 1 and sti % 2 == 1) else nc.gpsimd
                oeng.dma_start(out=d_out[st * P:(st + 1) * P, :], in_=ot)

        def consume(li, c, hgru_c):
            """out-projection for (li, c), eviction with bias, then the next
            layer's stats chain (or the final LN + logits for the last layer).
            """
            split = isinstance(hgru_c[0], list)  # final chunk: 256-wide scans
            sw = 256 if split else 512
            for m in range(KT):
                if li == L - 1 and c >= 1:
                    # vocab projection lags one chunk, interleaved two
                    # st-tiles per out-proj m-tile: the pso matmuls hide the
                    # psl slot-release latency (evictions lag on busy ACT/DVE)
                    logits_st(c - 1, 2 * m)
                    logits_st(c - 1, 2 * m + 1)
                ps = ps_mm.tile([P, SC], F32, name="pso", tag="psmm")
                for sub in range(SC // sw):
                    psl = ps[:, sub * sw:(sub + 1) * sw]
                    base = c * SC + sub * sw
                    for k in range(KT):
                        rhs = (hgru_c[k][sub] if split
                               else hgru_c[k][:, sub * sw:(sub + 1) * sw])
                        nc.tensor.matmul(
                            psl, wot[li, k][:, m * P:(m + 1) * P], rhs,
                            start=(k == 0), stop=False)
                    nc.tensor.matmul(psl, ident_sb,
                                     h[m][:, base:base + sw],
                                     start=False, stop=True)
                bcol = bout_sb[:, li * KT + m:li * KT + m + 1]
                nc.scalar.activation(h[m][:, c * SC:(c + 1) * SC], ps,
                                     AF.Identity, bias=bcol)
            if li < L - 1:
                stats_chain(li + 1, c)
            else:
                ln_stats_chain(c)

        # ---------------- flat software pipeline ----------------
        # warm the PE HAM window through the DMA-bound startup: dummy matmuls
        # chained to early DMA arrivals (one per arrival epoch, so ready real
        # matmuls aren't head-of-line blocked behind a waiting keep-alive)
        for k in range(KT):
            keepalive_mm(l1_av[k, 0][0])
        keepalive_mm(wot[0, 0])
        keepalive_mm(h[0])
        keepalive_mm(w1[1, 0])

        pend = None  # (li, c, hgru_c) awaiting consume
        for li in range(L):
            prev_scan = [None] * KT
            for c in range(NSC):
                if li == 0 and c >= 1:
                    load_chunk_inputs(c)
                hgru_c = produce(li, c)
                if pend is not None:
                    consume(*pend)
                if li == 0 and ka_q:
                    # filler matmuls between out-projection bursts keep HAM hot
                    for t in ka_q:
                        keepalive_mm(t)
                    ka_q.clear()
                pend = (li, c, hgru_c)
                if li == 0 and c == 1:
                    load_late_consts_1()
                elif li == 0 and c == 2:
                    load_late_consts_2()
        consume(*pend)
        for sti in range(SC // P):
            logits_st(NSC - 1, sti)

    nc.finalize()
    return nc


def _sigmoid(x):
    return 1.0 / (1.0 + np.exp(-x))


def _host_prep(inputs):
    """Precompute per-core device inputs (numpy, float64 internal)."""
    mm_np = _np_dt(MM_KIND)
    sc_np = _np_dt(SC_KIND)

    x = np.asarray(inputs["x"]).astype(np.int64)        # [B, S]
    emb = np.asarray(inputs["emb"]).astype(np.float64)  # [V, D]
    rms_w = np.asarray(inputs["rms_w"]).astype(np.float64)
    W_hg = np.asarray(inputs["W_hg"]).astype(np.float64)
    W_out = np.asarray(inputs["W_out"]).astype(np.float64)
    b_out = np.asarray(inputs["b_out"]).astype(np.float64)
    ln_w = np.asarray(inputs["ln_w"]).astype(np.float64)
    ln_b = np.asarray(inputs["ln_b"]).astype(np.float64)
    out_W = np.asarray(inputs["out_W"]).astype(np.float64)
    out_b = np.asarray(inputs["out_b"]).astype(np.float64)

    # layer-1 token tables
    r0 = 1.0 / np.sqrt((emb * emb).mean(-1, keepdims=True) + EPS_RMS)
    rms1 = emb * r0 * rms_w[0][None, :]
    hg1 = rms1 @ W_hg[0].T
    hid1, gate1 = hg1[:, :D], hg1[:, D:]
    z1 = _sigmoid(gate1)
    a_tab = _sigmoid(-gate1)                               # [V, D]
    v_tab = z1 * np.maximum(hid1 + 0.5, _sigmoid(hid1))    # [V, D]

    # folded weights
    W1 = np.stack([(W_hg[i] * rms_w[i][None, :]).T for i in range(1, L)])
    W1 = W1.reshape(L - 1, KT, P, 2 * D)
    WoT = np.stack([W_out[i].T for i in range(L)]).reshape(L, KT, P, D)
    W2 = (out_W * ln_w[None, :]).T                         # [D, V]
    cw = W2.sum(0)[None, :]                                # [1, V]
    b2 = out_b + out_W @ ln_b                              # [V]
    # b_out as per-partition columns: col (li*KT + m) = b_out[li, m*P:(m+1)*P]
    bout_cols = np.ascontiguousarray(
        b_out.reshape(L, KT, P).transpose(2, 0, 1).reshape(P, L * KT))

    common = {
        "W1": W1.astype(mm_np),
        "WoT": WoT.astype(mm_np),
        "W2": W2.reshape(KT, P, V).astype(mm_np),
        "boutC": bout_cols.astype(np.float32),
        "b2rep": np.tile(b2[None, :].astype(np.float32), (P, 1)),
        "cw": cw.astype(mm_np),
        "onesk": np.ones((P, 1), mm_np),
        "onesr": np.ones((1, P), mm_np),
        "ident": np.eye(P, dtype=np.float32).astype(mm_np),
    }
    in_maps = []
    for b in range(B):
        tok = x[b]
        m = dict(common)
        m["h0T"] = np.ascontiguousarray(emb[tok].T).astype(mm_np)
        m["a1T"] = np.ascontiguousarray(a_tab[tok].T).astype(sc_np)
        m["v1T"] = np.ascontiguousarray(v_tab[tok].T).astype(sc_np)
        in_maps.append(m)
    return in_maps


def _get_nc(b2_zero):
    key = ("nc", b2_zero)
    if key not in _cache:
        _cache[key] = _build_nc(b2_zero)
    return _cache[key]


def kernel(**inputs):
    from concourse.bass_utils import run_bass_kernel_spmd

    b2 = (np.asarray(inputs["out_b"], np.float64)
          + np.asarray(inputs["out_W"], np.float64)
          @ np.asarray(inputs["ln_b"], np.float64))
    nc = _get_nc(b2_zero=bool(np.all(b2 == 0.0)))
    in_maps = _host_prep(inputs)
    res = run_bass_kernel_spmd(nc, in_maps, core_ids=list(range(N_CORES)),
                               trace=bool(int(os.environ.get("EM_TRACE", "0"))))
    _cache["last_result"] = res
    out = np.stack([r["out"] for r in res.results], axis=0)
    return out.astype(np.float32)
